# revision 10
# baseline (speedup 1.0000x reference)
"""Trainium2 Bass kernel for nn_AttentionBlock (sparse 7x7 windowed per-channel attention).

Semantics (validated vs reference): the torch-faithful scrambled reshape makes this,
in original coordinates, a per-(b, c, h, w-block-of-16) attention:
  logits[kh,kw] = sum_{d<16} q[c,h,16w0+d] * kpad[c,h+kh,16w0+d+kw] + qsum*(rh[kh]+rw[kw])
  out[c,h,16w0+d] = relu( sum_{kh,kw} softmax(logits)[kh,kw] * vpad[c,h+kh,16w0+d+kw] )
where kpad/vpad = conv1x1(x)+bias inside the image and exactly bias in the pad border
(conv of zero-padded x reproduces this).

Sharding: pure data parallel over 8 cores: core j -> batch j//2, image rows
[48*(j%2), 48*(j%2)+48). Each core packs its 48 rows as 2 partition-groups of 24 rows
(partition = c + 64*g).

This version (vs the 322us DVE-tree baseline) keeps only the irreducible window
products on DVE and moves every reduction to the (previously idle) PE as
identity-weight accumulating matmuls into PSUM:
  - q/k/v are stored TRANSPOSED (w-major, h innermost). The softmax weights then
    multiply v with a stride-0 *middle* (d) axis which keeps the DVE 2x 16-bit
    mode - the baseline's ~100us ACT weight d-expansion is gone entirely.
    Odd-kh taps read 1-row-shifted copies (k2o/v2o) to keep reads 4B-aligned.
  - qk d-reduction: per tap-pair, 16 d-slice matmuls (lhsT=I128) accumulate into
    a PSUM tile; a 17th matmul adds the precomputed qs*rel_t term. ACT exp reads
    the PSUM logits directly and writes bf16 exp(l); bf16's range makes softmax
    max-subtraction and weight normalization unnecessary (1/den is folded into
    the final relu-scale).
  - softmax denominator: 49 accumulating 144-col matmuls over the exp slices.
  - AV: DVE bf16 products, then per tap 6 bank-chunk identity matmuls accumulate
    into a persistent PSUM accumulator (fp32 - more accurate than the baseline's
    fp16 add chain).
  - Final: per chunk, out = max(acc,0) * rden (scalar_tensor_tensor, rden
    broadcast over d), DMA out; the host de-transposes.
"""

import sys
import os

sys.path.insert(0, "/opt/trn_rl_repo")

import numpy as np

B, C, H, W = 4, 64, 96, 96
KS, NH = 7, 4
HALO = (KS - 1) // 2          # 3
NCORES = 8
RPC = H // 2                  # 48 rows per core
G = 2                         # partition groups per core
RPG = RPC // G                # 24 rows per group
KR = RPG + KS - 1             # 30 k/v rows per group
XR = RPC + KS - 1             # 54 x rows per core
WE = W + KS - 1               # 102 extended cols
NB = W // 16                  # 6 w-blocks
ND = 16                       # d values per block
NS = NB * RPG                 # 144 sites per partition (wb-major, h innermost)
NT = KS * KS                  # 49 taps
ACC_CH = (432, 432, 432, 432, 432, 144)  # PSUM accumulator bank chunks (3,3,3,3,3,1 d-planes)

_cache = {}


def _build():
    import concourse.bacc as bacc
    import concourse.bass as bass
    import concourse.tile as tile
    from concourse import mybir

    f32 = mybir.dt.float32
    f16 = mybir.dt.float16
    bf16 = mybir.dt.bfloat16
    Act = mybir.ActivationFunctionType
    Alu = mybir.AluOpType

    nc = bacc.Bacc(
        "TRN2",
        target_bir_lowering=False,
        debug=False,
        enable_asserts=False,
        num_devices=NCORES,
    )

    xc_d = nc.dram_tensor("xc", [C + 1, WE, XR], f16, kind="ExternalInput").ap()
    wq_d = nc.dram_tensor("wq", [C + 1, C], f16, kind="ExternalInput").ap()
    wk_d = nc.dram_tensor("wk", [C + 1, C], f16, kind="ExternalInput").ap()
    wv_d = nc.dram_tensor("wv", [C + 1, C], f16, kind="ExternalInput").ap()
    rel_d = nc.dram_tensor("relv", [NT], f32, kind="ExternalInput").ap()
    idf_d = nc.dram_tensor("idf", [128, 128], f16, kind="ExternalInput").ap()
    idb_d = nc.dram_tensor("idb", [128, 128], bf16, kind="ExternalInput").ap()
    # out columns are the AV accumulator layout (d, wb, h); host de-transposes
    out_d = nc.dram_tensor("outp", [2 * C, ND, NB, RPG], f32, kind="ExternalOutput").ap()

    from contextlib import ExitStack

    with tile.TileContext(nc) as tc:
        with ExitStack() as stk:
            wp = stk.enter_context(tc.tile_pool(name="wpool", bufs=1))
            mp = stk.enter_context(tc.tile_pool(name="main", bufs=1))
            tp = stk.enter_context(tc.tile_pool(name="tmp", bufs=2))
            pp_cm = tc.tile_pool(name="psumP", bufs=2, space=bass.MemorySpace.PSUM)
            pp = pp_cm.__enter__()
            xp_cm = tc.tile_pool(name="xpool", bufs=1)
            xp = xp_cm.__enter__()

            # ---- loads ----
            wq = wp.tile([C + 1, C], f16)
            wk = wp.tile([C + 1, C], f16)
            wv = wp.tile([C + 1, C], f16)
            idf = wp.tile([128, 128], f16)
            idb = wp.tile([128, 128], bf16)
            nc.sync.dma_start(out=wk, in_=wk_d)
            nc.sync.dma_start(out=wq, in_=wq_d)
            nc.sync.dma_start(out=wv, in_=wv_d)
            nc.sync.dma_start(out=idf, in_=idf_d)
            nc.sync.dma_start(out=idb, in_=idb_d)
            xc = xp.tile([C + 1, WE, XR], f16)
            nc.sync.dma_start(out=xc[:, : WE // 2, :], in_=xc_d[:, : WE // 2, :])
            nc.sync.dma_start(out=xc[:, WE // 2 :, :], in_=xc_d[:, WE // 2 :, :])
            relsb = wp.tile([128, NT], f32)
            nc.gpsimd.dma_start(
                out=relsb, in_=rel_d.unsqueeze(0).broadcast_to([128, NT])
            )

            # ---- persistent tensors (all transposed: w-major, h innermost) ----
            k2 = mp.tile([128, WE, KR], f16)
            k2o = mp.tile([128, WE, KR], f16)    # shifted 1 row (h+1) for odd kh
            v2 = mp.tile([128, WE, KR], bf16)
            v2o = mp.tile([128, WE, KR], bf16)
            q2 = mp.tile([128, W, RPG], f16)
            qs16 = mp.tile([128, NS], f16)
            qsrel = mp.tile([128, NT, NS], f16)
            Abf = mp.tile([128, NT, NS], bf16)   # exp(logits), unnormalized
            rden = mp.tile([128, NS], f32)
            outsb = mp.tile([128, ND, NB, RPG], f32)

            # ---- projections: k (+shifted copy), q, v (+shifted copy) ----
            KCH = 6
            kcw = WE // KCH  # 17 w'-cols per chunk
            kn = kcw * KR    # 510

            def kv_proj(dst, dsto, wgt):
                for ci in range(KCH):
                    ps = pp.tile([128, 512], f32, tag="ps_kv", name="ps")
                    for g in range(G):
                        rhs = xc[:, ci * kcw : (ci + 1) * kcw, RPG * g : RPG * g + KR]
                        nc.tensor.matmul(
                            ps[64 * g : 64 * g + 64, :kn], wgt, rhs,
                            start=True, stop=True,
                        )
                    psv = ps[:, :kn].rearrange("p (a b) -> p a b", b=KR)
                    nc.scalar.copy(dst[:, ci * kcw : (ci + 1) * kcw, :], psv)
                    nc.scalar.copy(
                        dsto[:, ci * kcw : (ci + 1) * kcw, : KR - 1],
                        psv[:, :, 1:KR],
                    )

            kv_proj(k2, k2o, wk)

            QCH = 6
            qcw = W // QCH  # 16
            qn = qcw * RPG  # 384
            for ci in range(QCH):
                ps = pp.tile([128, 512], f32, tag="ps_kv", name="psq")
                for g in range(G):
                    rhs = xc[
                        :, HALO + ci * qcw : HALO + (ci + 1) * qcw,
                        HALO + RPG * g : HALO + RPG * g + RPG,
                    ]
                    nc.tensor.matmul(
                        ps[64 * g : 64 * g + 64, :qn], wq, rhs, start=True, stop=True
                    )
                nc.scalar.copy(
                    q2[:, ci * qcw : (ci + 1) * qcw, :],
                    ps[:, :qn].rearrange("p (a b) -> p a b", b=RPG),
                )

            kv_proj(v2, v2o, wv)

            # release x + projection-psum pools
            xp_cm.__exit__(None, None, None)
            pp_cm.__exit__(None, None, None)
            pq_cm = tc.tile_pool(name="psumQ", bufs=1, space=bass.MemorySpace.PSUM)
            pq = pq_cm.__enter__()

            # ---- qs = sum_d q (DVE fp16 pairwise tree over d) ----
            qv = q2.rearrange("p (wb d) h -> p wb d h", d=ND)  # [p, wb, d, h]
            t1 = tp.tile([128, NB, 8, RPG], f16, tag="qs1", bufs=1)
            nc.vector.tensor_add(t1, qv[:, :, 0:8], qv[:, :, 8:16])
            t2 = tp.tile([128, NB, 4, RPG], f16, tag="qs2", bufs=1)
            nc.vector.tensor_add(t2, t1[:, :, 0:4], t1[:, :, 4:8])
            t3 = tp.tile([128, NB, 2, RPG], f16, tag="qs3", bufs=1)
            nc.vector.tensor_add(t3, t2[:, :, 0:2], t2[:, :, 2:4])
            nc.vector.tensor_add(
                qs16.rearrange("p (wb h) -> p wb h", h=RPG), t3[:, :, 0], t3[:, :, 1]
            )
            # qsrel[:, t, :] = qs * rel_t  (DVE tensor_scalar, 4x mode)
            for t in range(NT):
                nc.vector.tensor_scalar_mul(qsrel[:, t, :], qs16, relsb[:, t : t + 1])

            # ---- qk: DVE products; PE d-reduce + rel into PSUM; ACT exp -> bf16 ----
            den_ps = pq.tile([128, NS], f32, tag="den", name="den", bufs=1)
            den_n = [0]

            def den_push(t0, nt):
                for t in range(t0, t0 + nt):
                    nc.tensor.matmul(
                        den_ps, idb, Abf[:, t, :],
                        start=(t == 0), stop=(t == NT - 1),
                    )

            for kh in range(KS):
                ksrc, koff = (k2, kh) if kh % 2 == 0 else (k2o, kh - 1)
                prod = tp.tile(
                    [128, ND, KS, NB, RPG], f16, tag="prod", name="prod", bufs=2
                )
                for kw in range(KS):
                    kview = ksrc[:, kw : kw + W, koff : koff + RPG].rearrange(
                        "p (wb d) h -> p wb d h", d=ND
                    )
                    nc.vector.tensor_mul(
                        prod[:, :, kw].transpose([0, 2, 1, 3]), qv, kview
                    )
                # PE: per tap pair/single, 16 d-slices + qs*rel slice
                for kw0, nkw in ((0, 2), (2, 2), (4, 2), (6, 1)):
                    t0 = kh * KS + kw0
                    ncol = nkw * NS
                    if nkw == 2:
                        aps = pq.tile([128, 2 * NS], f32, tag="apair", name="ap", bufs=3)
                    else:
                        aps = pq.tile([128, NS], f32, tag="asing", name="as", bufs=2)
                    for d in range(ND):
                        rhs = prod[:, d, kw0 : kw0 + nkw].rearrange(
                            "p a b c -> p (a b c)"
                        )
                        nc.tensor.matmul(
                            aps[:, :ncol], idf, rhs, start=(d == 0), stop=False
                        )
                    rrhs = qsrel[:, t0 : t0 + nkw, :].rearrange("p a b -> p (a b)")
                    nc.tensor.matmul(aps[:, :ncol], idf, rrhs, start=False, stop=True)
                    nc.scalar.activation(
                        Abf[:, t0 : t0 + nkw, :],
                        aps[:, :ncol].rearrange("p (t s) -> p t s", s=NS),
                        Act.Exp,
                    )
                den_push(kh * KS, KS)

            # den -> 1/den, then release the qk psum pool for the AV accumulators
            nc.vector.reciprocal(rden, den_ps)
            pq_cm.__exit__(None, None, None)
            pa = stk.enter_context(
                tc.tile_pool(name="psumA", bufs=1, space=bass.MemorySpace.PSUM)
            )

            # ---- AV: DVE bf16 products; PE accumulating identity matmuls ----
            accs = [
                pa.tile([128, ACC_CH[j]], f32, tag=f"acc{j}", name=f"acc{j}", bufs=1)
                for j in range(6)
            ]
            for kh in range(KS):
                vsrc, voff = (v2, kh) if kh % 2 == 0 else (v2o, kh - 1)
                prodv = tp.tile(
                    [128, KS, ND, NB, RPG], bf16, tag="prod", name="prodv", bufs=2
                )
                for kw in range(KS):
                    t = kh * KS + kw
                    aview = (
                        Abf[:, t, :]
                        .rearrange("p (wb h) -> p wb h", h=RPG)
                        .unsqueeze(2)
                        .broadcast_to([128, NB, ND, RPG])
                    )
                    vview = vsrc[:, kw : kw + W, voff : voff + RPG].rearrange(
                        "p (wb d) h -> p wb d h", d=ND
                    )
                    nc.vector.tensor_mul(
                        prodv[:, kw].transpose([0, 2, 1, 3]), aview, vview
                    )
                    flat = prodv[:, kw].rearrange("p a b c -> p (a b c)")
                    c0 = 0
                    for j, cw in enumerate(ACC_CH):
                        nc.tensor.matmul(
                            accs[j], idb, flat[:, c0 : c0 + cw],
                            start=(t == 0), stop=(t == NT - 1),
                        )
                        c0 += cw

            # ---- final: out = max(acc, 0) * rden (rden broadcast over d) ----
            rv3 = (
                rden.rearrange("p (wb h) -> p wb h", h=RPG)
                .unsqueeze(1)
                .broadcast_to([128, 3, NB, RPG])
            )
            rv1 = (
                rden.rearrange("p (wb h) -> p wb h", h=RPG)
                .unsqueeze(1)
                .broadcast_to([128, 1, NB, RPG])
            )
            for j, cw in enumerate(ACC_CH):
                nd = cw // NS  # d-planes in this chunk
                accv = accs[j].rearrange("p (d wb h) -> p d wb h", wb=NB, h=RPG)
                outv = outsb[:, 3 * j : 3 * j + nd]
                nc.vector.scalar_tensor_tensor(
                    outv, accv, 0.0, rv3 if nd == 3 else rv1, Alu.max, Alu.mult
                )
                nc.sync.dma_start(
                    out=out_d[:, 3 * j : 3 * j + nd], in_=outv
                )

    nc.compile()
    return nc


def _get_nc():
    if "nc" not in _cache:
        _cache["nc"] = _build()
    return _cache["nc"]


def _prep_inputs(inputs):
    """Host-side shard prep. Returns list of 8 in_maps."""
    import ml_dtypes

    x = np.ascontiguousarray(np.asarray(inputs["input_x"], dtype=np.float32))
    qw = np.asarray(inputs["q_w"], np.float32)
    qb = np.asarray(inputs["q_b"], np.float32)
    kw_ = np.asarray(inputs["k_w"], np.float32)
    kb = np.asarray(inputs["k_b"], np.float32)
    vw = np.asarray(inputs["v_w"], np.float32)
    vb = np.asarray(inputs["v_b"], np.float32)
    rh = np.asarray(inputs["rel_h"], np.float32).sum(0)[:, 0]  # (7,)
    rw = np.asarray(inputs["rel_w"], np.float32).sum(0)[0, :]  # (7,)

    wq = np.concatenate([qw.T, qb[None, :]], axis=0).astype(np.float16)  # (65, 64)
    wk = np.concatenate([kw_.T, kb[None, :]], axis=0).astype(np.float16)
    wv = np.concatenate([vw.T, vb[None, :]], axis=0).astype(np.float16)
    relv = (rh[:, None] + rw[None, :]).reshape(-1).astype(np.float32)  # (49,)
    idf = np.eye(128, dtype=np.float16)
    idb = np.eye(128, dtype=np.float32).astype(ml_dtypes.bfloat16)

    # padded x with ones channel: (B, 65, 102, 102)
    xpad = np.zeros((B, C + 1, H + 2 * HALO, W + 2 * HALO), np.float16)
    xpad[:, :C, HALO : HALO + H, HALO : HALO + W] = x
    xpad[:, C, :, :] = 1.0

    in_maps = []
    for j in range(NCORES):
        b = j // 2
        r0 = RPC * (j % 2)
        # transposed: [c, w', h']
        xcT = np.ascontiguousarray(
            xpad[b, :, r0 : r0 + XR, :].transpose(0, 2, 1)
        )  # (65, 102, 54)
        in_maps.append(
            {"xc": xcT, "wq": wq, "wk": wk, "wv": wv, "relv": relv,
             "idf": idf, "idb": idb}
        )
    return in_maps


def _assemble(results):
    """results: list of 8 dicts with 'outp' (128, 16, 6, 24) -> (B, C, H, W)."""
    y = np.empty((B, C, H, W), np.float32)
    for j in range(NCORES):
        o = results[j]["outp"]  # [c+64g, d, wb, h]
        b = j // 2
        r0 = RPC * (j % 2)
        for g in range(G):
            blk = o[64 * g : 64 * g + 64]  # (64, 16, 6, 24)
            y[b, :, r0 + RPG * g : r0 + RPG * (g + 1), :] = (
                blk.transpose(0, 3, 2, 1).reshape(C, RPG, W)
            )
    return y


def _install_ntff_hook():
    """Register the axon NTFF profiling hook (the image lacks antenv.axon_hooks)."""
    import types
    import antenv

    if "antenv.axon_hooks" in sys.modules:
        return
    mod = types.ModuleType("antenv.axon_hooks")
    _state = {"hook": None}
    mod.set_axon_ntff_profile_hook = lambda h: _state.__setitem__("hook", h)
    mod.get_axon_ntff_profile_hook = lambda: _state["hook"]
    sys.modules["antenv.axon_hooks"] = mod
    antenv.axon_hooks = mod
    from trn_agent_boot.trn_boot import _ntff_profile_via_ctypes

    mod.set_axon_ntff_profile_hook(_ntff_profile_via_ctypes("/opt/axon/libaxon_pjrt.so"))
    # avoid S3 artifact uploads in-container
    from concourse import bass_utils

    bass_utils.upload_artifacts = lambda tmpdir: tmpdir


def kernel(**inputs) -> np.ndarray:
    from concourse import bass_utils

    nc = _get_nc()
    in_maps = _prep_inputs(inputs)
    trace = bool(int(os.environ.get("KERNEL_TRACE", "0")))
    kw = {}
    if trace:
        _install_ntff_hook()
        kw["tmpdir"] = os.environ.get("KERNEL_TRACE_DIR") or None
    res = bass_utils.run_bass_kernel_spmd(
        nc, in_maps, core_ids=list(range(NCORES)), trace=trace, **kw
    )
    _cache["last_result"] = res
    return _assemble(res.results)


def kernel_sim(inputs, cores=(0,)):
    """CoreSim-based check (no hardware). Returns partial output dict {core: outp}."""
    from concourse.bass_interp import CoreSim

    nc = _get_nc()
    in_maps = _prep_inputs(inputs)
    outs = {}
    for j in cores:
        sim = CoreSim(nc, trace=False, require_finite=True, require_nnan=True)
        for name, arr in in_maps[j].items():
            sim.tensor(name)[:] = arr
        sim.simulate(check_with_hw=False)
        outs[j] = np.array(sim.tensor("outp"))
    return outs
